# revision 1
# baseline (speedup 1.0000x reference)
"""Trainium2 Bass kernel for nn_Encoder_MLP (embedding gather + sum + 2-layer MLP tail).

Reference computation:
    x = where(gate_seq < 0, A, gate_seq)            # [B, T]   (inputs never negative)
    Wr = W1.reshape(T, V, HID)
    h  = Wr[arange(T)[None,:], x].sum(axis=1) + b1  # [B, HID]  gather B*T rows, sum over T
    h  = relu(h); h = relu(h @ W2 + b2); out = h @ W3 + b3

Sharding (8 cores): shard the T (position) axis 8-ways. Core m holds the
W1 rows for positions [32m, 32m+32) = [131072, 256] in bf16 (64 MB; bf16
halves gather bytes and host staging, desc-gen cost is unchanged). Each core
gathers its 64*32 = 2048 rows with 4 dma_gather calls (int16 indices address
a 32768-row window = 8 positions; measured cost ~1.8us fixed + 6ns/idx per
call, serial on Q7, so 4x512 is the optimal granularity), reduces them to a
[64, 256] partial sum (bf16 DVE folds + one bf16 mask matmul per window into
f32 PSUM), then one 8-core bf16 ReduceScatter yields batches [8m, 8m+8) on
core m, where the bf16 MLP tail runs. Host concatenates the per-core
[8, 256] outputs. Each gather runs on its own SWDGE queue (single-queue
ring-reclaim backpressure made calls serialize at ~4.7us each; with 4 queues
calls 2-4 cost ~70ns). The first collective starts at max(trigger,
collectives-init barrier end ~33.5us) + ~11.5us ncfw wake; with the RS
trigger at ~32us the whole gather+reduce pipeline is hidden behind that
floor on a fresh runtime.

Index layout (device gathers g[P, slot, :] = W1win[idx_i], i = slot*128 + P):
  idx list position i lives at idx_tile[i % 16, i // 16] (16-partition wrap,
  replicated x8 for the 8 Q7 cores). We order indices so gathered partition P
  always holds batch P % 64 and (slot, P//64) enumerate the 8 positions of a
  window: value(p16, scol) = (scol//4)*4096 + gate[16*(scol%4) + p16, 8w + scol//4].
  The +u*4096 rebase is done on device (ubias const + DVE add); the host only
  permutes/retypes gate_seq (value-independent layout marshaling).
"""

import sys

import numpy as np

if "/opt/trn_rl_repo" not in sys.path:
    sys.path.insert(0, "/opt/trn_rl_repo")

B = 64
T = 256
V = 4096
HID = 256
OUT = 256
NCORES = 8
TPC = T // NCORES          # positions per core = 32
WIN_POS = 8                # positions per gather window (int16 limit: 8*4096 = 32768 rows)
NWIN = TPC // WIN_POS      # 4 windows per core
WIN_ROWS = WIN_POS * V     # 32768
SHARD_ROWS = TPC * V       # 131072
NIDX = B * WIN_POS         # 512 indices per window
BPC = B // NCORES          # batches per core after ReduceScatter = 8

_CACHE = {}


def _host_consts():
    # ubias[p, f] = ((f%32)//4) * 4096  (int16; per-free-column rebase)
    f = np.arange(NWIN * 32)
    ubias = np.broadcast_to(((f % 32) // 4) * V, (128, NWIN * 32)).astype(np.int16)
    # mask[P, b] = 1 if P % 64 == b
    P = np.arange(128)[:, None]
    import ml_dtypes
    mask = (P % B == np.arange(B)[None, :]).astype(ml_dtypes.bfloat16)
    eye8 = np.eye(8, dtype=np.float32)
    eye8b = np.eye(8, dtype=ml_dtypes.bfloat16)  # noqa: F841  (bf16 eye kept for h2 transposes)
    return np.ascontiguousarray(ubias), np.ascontiguousarray(mask), eye8, eye8b


def _build_nc():
    import concourse.bacc as bacc
    import concourse.mybir as mybir
    import concourse.tile as tile

    f32 = mybir.dt.float32
    bf16 = mybir.dt.bfloat16
    i16 = mybir.dt.int16
    Relu = mybir.ActivationFunctionType.Relu
    add = mybir.AluOpType.add

    ubias_np, mask_np, eye8_np, eye8b_np = _host_consts()

    nc = bacc.Bacc(
        "TRN2",
        target_bir_lowering=False,
        debug=False,
        num_devices=NCORES,
        num_swdge_queues=4,
    )

    gate_prep_d = nc.dram_tensor("gate_prep", [128, NWIN * 32], i16, kind="ExternalInput")
    w1_d = nc.dram_tensor("w1", [SHARD_ROWS, HID], bf16, kind="ExternalInput")
    w2_d = nc.dram_tensor("w2", [HID, HID], bf16, kind="ExternalInput")
    w3_d = nc.dram_tensor("w3", [HID, OUT], bf16, kind="ExternalInput")
    b1_d = nc.dram_tensor("b1t", [128, 2], f32, kind="ExternalInput")
    b2_d = nc.dram_tensor("b2", [1, HID], bf16, kind="ExternalInput")
    b3_d = nc.dram_tensor("b3", [1, OUT], bf16, kind="ExternalInput")
    out_d = nc.dram_tensor("out", [BPC, OUT], f32, kind="ExternalOutput")

    ubias_d = nc.inline_tensor(ubias_np, name="ubias_const")
    mask_d = nc.inline_tensor(mask_np, name="mask_const")
    eyeb_d = nc.inline_tensor(eye8b_np, name="eyeb_const")
    eye_d = nc.inline_tensor(eye8_np, name="eye_const")

    # Issue the mlp ucode library load before any Tile-scheduled work so the
    # ~10us Q7 library fetch overlaps the NEFF prologue instead of stalling
    # the first dma_gather until ~17us.
    from concourse import library_config

    nc.gpsimd.load_library(library_config.mlp)

    with tile.TileContext(nc) as tc:
        with (
            tc.tile_pool(name="const", bufs=1) as const,
            tc.tile_pool(name="gat", bufs=1) as gat,
            tc.tile_pool(name="work", bufs=2) as work,
            tc.tile_pool(name="psum", bufs=1, space="PSUM") as psum,
            tc.tile_pool(name="dram", bufs=1, space="DRAM") as dram,
        ):
            # ---- critical path: indices ----
            gp = const.tile([128, NWIN * 32], i16, tag="gp")
            nc.sync.dma_start(gp[:], gate_prep_d[:])
            ub = const.tile([128, NWIN * 32], i16, tag="ub")
            nc.sync.dma_start(ub[:], ubias_d[:])
            idx = const.tile([128, NWIN * 32], i16, tag="idx")
            nc.vector.tensor_tensor(idx[:], gp[:], ub[:], add)

            # ---- gathers (SWDGE custom ucode, 8-way Q7 desc-gen) ----
            g_tiles = []
            for w in range(NWIN):
                g = gat.tile([128, NIDX // 128, HID], bf16, tag=f"g{w}")
                nc.gpsimd.dma_gather(
                    g[:],
                    w1_d[w * WIN_ROWS : (w + 1) * WIN_ROWS, :],
                    idx[:, w * 32 : (w + 1) * 32],
                    NIDX,
                    NIDX,
                    HID,
                    queue_num=w,
                )
                g_tiles.append(g)

            # ---- constants / weights preload (no deps; fills DMA idle time) ----
            mask_sb = const.tile([128, B], bf16, tag="mask")
            nc.scalar.dma_start(mask_sb[:], mask_d[:])
            eyeb_sb = const.tile([8, 8], bf16, tag="eyeb")
            nc.scalar.dma_start(eyeb_sb[:], eyeb_d[:])
            eye_sb = const.tile([8, 8], f32, tag="eye")
            nc.scalar.dma_start(eye_sb[:], eye_d[:])
            w2_sb = const.tile([128, 2, HID], bf16, tag="w2")
            nc.scalar.dma_start(w2_sb[:], w2_d[:, :].rearrange("(k p) n -> p k n", p=128))
            w3_sb = const.tile([128, 2, OUT], bf16, tag="w3")
            nc.scalar.dma_start(w3_sb[:], w3_d[:, :].rearrange("(k p) n -> p k n", p=128))
            b1_sb = const.tile([128, 2], f32, tag="b1")
            nc.scalar.dma_start(b1_sb[:], b1_d[:])
            b2_sb = const.tile([1, HID], bf16, tag="b2")
            nc.scalar.dma_start(b2_sb[:], b2_d[:])
            b3_sb = const.tile([1, OUT], bf16, tag="b3")
            nc.scalar.dma_start(b3_sb[:], b3_d[:])
            ones8 = const.tile([1, BPC], bf16, tag="ones8")
            nc.vector.memset(ones8[:], 1.0)

            # ---- per-window fold (DVE) + mask matmul accumulate (PE) ----
            # Note: a 2-chunk pipelined RS was tried and reverted — the first
            # collective start is pinned at max(doorbell, barrier_end)+~11.5us
            # (ncfw bootstrap ~21us + core skew), so an earlier doorbell buys
            # nothing and a second RS only appends its duration.
            psum_part = psum.tile([B, HID], f32, tag="part")
            for w, g in enumerate(g_tiles):
                u1 = work.tile([128, 2, HID], bf16, tag="u1")
                nc.vector.tensor_add(u1[:], g[:, 0:2, :], g[:, 2:4, :])
                s = work.tile([128, HID], bf16, tag="s")
                nc.vector.tensor_add(s[:], u1[:, 0, :], u1[:, 1, :])
                nc.tensor.matmul(
                    psum_part[:], mask_sb[:], s[:], start=(w == 0), stop=(w == NWIN - 1)
                )

            part_sb = work.tile([B, HID], f32, tag="part_sb")
            nc.vector.tensor_copy(part_sb[:], psum_part[:])

            # ---- cross-core ReduceScatter: [64,256] -> [8,256] on core m ----
            # (AllGather + local 8-way reduce was tried: AG Mesh dur ~11.5us
            # vs RS RDH ~12.6us, but the 256KB strided gather-in of the AG
            # output costs ~3.7us and the redundant [64,..] tail adds more -
            # RS' post-collective path is ~3us shorter net. Reverted.)
            cc_in = dram.tile([B, HID], f32, tag="cc_in")
            cc_out = dram.tile([BPC, HID], f32, tag="cc_out")
            nc.sync.dma_start(cc_in[:], part_sb[:])
            nc.gpsimd.collective_compute(
                "ReduceScatter",
                add,
                replica_groups=[list(range(NCORES))],
                ins=[cc_in[:].opt()],
                outs=[cc_out[:].opt()],
            )
            h_sb = work.tile([BPC, HID], f32, tag="h")
            nc.sync.dma_start(h_sb[:], cc_out[:])

            # ---- tail MLP on [8, 256] shard ----
            # hT = transpose(h) (PE), relu(hT + b1T) per 128-chunk
            hTr = []
            for m in range(2):
                p_hT = psum.tile([128, BPC], f32, tag=f"p_hT{m}")
                nc.tensor.transpose(p_hT[:], h_sb[:, m * 128 : (m + 1) * 128], eye_sb[:])
                t = work.tile([128, BPC], bf16, tag=f"hTr{m}")
                nc.scalar.activation(t[:], p_hT[:], Relu, bias=b1_sb[:, m : m + 1])
                hTr.append(t)

            # h2 = relu(hTr.T @ W2 + b2)   -> [8, 256]
            p_h2 = psum.tile([BPC, HID], f32, tag="p_h2")
            nc.tensor.matmul(p_h2[:], hTr[0][:], w2_sb[:, 0, :], start=True, stop=False)
            nc.tensor.matmul(p_h2[:], hTr[1][:], w2_sb[:, 1, :], start=False, stop=False)
            nc.tensor.matmul(p_h2[:], ones8[:], b2_sb[:], start=False, stop=True)
            h2_sb = work.tile([BPC, HID], bf16, tag="h2")
            nc.scalar.activation(h2_sb[:], p_h2[:], Relu)

            # out = h2 @ W3 + b3          -> [8, 256]
            h2T = []
            for m in range(2):
                p_h2T = psum.tile([128, BPC], bf16, tag=f"p_h2T{m}")
                nc.tensor.transpose(p_h2T[:], h2_sb[:, m * 128 : (m + 1) * 128], eyeb_sb[:])
                t = work.tile([128, BPC], bf16, tag=f"h2T{m}")
                nc.vector.tensor_copy(t[:], p_h2T[:])
                h2T.append(t)
            p_o = psum.tile([BPC, OUT], f32, tag="p_o")
            nc.tensor.matmul(p_o[:], h2T[0][:], w3_sb[:, 0, :], start=True, stop=False)
            nc.tensor.matmul(p_o[:], h2T[1][:], w3_sb[:, 1, :], start=False, stop=False)
            nc.tensor.matmul(p_o[:], ones8[:], b3_sb[:], start=False, stop=True)
            out_sb = work.tile([BPC, OUT], f32, tag="out_sb")
            nc.vector.tensor_copy(out_sb[:], p_o[:])
            nc.sync.dma_start(out_d[:], out_sb[:])

    nc.compile()
    return nc


def get_nc():
    if "nc" not in _CACHE:
        _CACHE["nc"] = _build_nc()
    return _CACHE["nc"]


def make_in_maps(gate_seq, W1, b1, W2, b2, W3, b3):
    """Shard/marshal the full inputs into per-core input maps (values untouched:
    pure slicing, transposition, retyping and tiling)."""
    gate_seq = np.asarray(gate_seq)
    import ml_dtypes

    W1 = np.ascontiguousarray(np.asarray(W1).astype(ml_dtypes.bfloat16))
    W2 = np.ascontiguousarray(np.asarray(W2).astype(ml_dtypes.bfloat16))
    W3 = np.ascontiguousarray(np.asarray(W3).astype(ml_dtypes.bfloat16))
    b1 = np.asarray(b1, dtype=np.float32)
    b2 = np.asarray(b2, dtype=np.float32)
    b3 = np.asarray(b3, dtype=np.float32)

    b1t = np.ascontiguousarray(b1.reshape(2, 128).T)  # b1t[p, m] = b1[m*128 + p]
    b2r = np.ascontiguousarray(b2[None, :].astype(ml_dtypes.bfloat16))
    b3r = np.ascontiguousarray(b3[None, :].astype(ml_dtypes.bfloat16))

    # index-layout permutation (see module docstring)
    p16 = np.arange(16)[:, None]                     # [16, 1]
    f = np.arange(NWIN * 32)[None, :]                # [1, 128]
    w = f // 32
    sp = f % 32
    b_idx = (sp % 4) * 16 + p16                      # [16, 128]
    t_idx = np.broadcast_to(w * WIN_POS + sp // 4, b_idx.shape)

    in_maps = []
    for m in range(NCORES):
        gs = gate_seq[:, m * TPC : (m + 1) * TPC]    # [64, 32]
        A = gs[b_idx, t_idx].astype(np.int16)        # [16, 128]
        gate_prep = np.ascontiguousarray(np.tile(A, (8, 1)))  # [128, 128]
        w1_shard = W1[m * SHARD_ROWS : (m + 1) * SHARD_ROWS]
        in_maps.append(
            {
                "gate_prep": gate_prep,
                "w1": w1_shard,
                "w2": W2,
                "w3": W3,
                "b1t": b1t,
                "b2": b2r,
                "b3": b3r,
            }
        )
    return in_maps


def run(inputs, trace=False, **spmd_kwargs):
    from concourse.bass_utils import run_bass_kernel_spmd

    nc = get_nc()
    in_maps = make_in_maps(**inputs)
    res = run_bass_kernel_spmd(
        nc, in_maps, core_ids=list(range(NCORES)), trace=trace, **spmd_kwargs
    )
    out = np.concatenate([r["out"] for r in res.results], axis=0)
    return out, res


def kernel(**inputs) -> np.ndarray:
    out, _ = run(inputs, trace=False)
    return out



# revision 4
# speedup vs baseline: 2.9523x; 2.9523x over previous
"""Trainium2 Bass kernel for nn_Encoder_MLP (embedding gather + sum + 2-layer MLP tail).

Reference computation:
    x = where(gate_seq < 0, A, gate_seq)            # [B, T]   (inputs never negative)
    Wr = W1.reshape(T, V, HID)
    h  = Wr[arange(T)[None,:], x].sum(axis=1) + b1  # [B, HID]  gather B*T rows, sum over T
    h  = relu(h); h = relu(h @ W2 + b2); out = h @ W3 + b3

Sharding (8 cores): data-parallel over the batch axis, W1 fully replicated
(bf16, 512 MB/core in HBM). Core m owns batches [8m, 8m+8) and gathers all
T=256 positions for them: 2048 rows via 32 dma_gather calls (the int16 index
limit caps one call's window at 32768 rows = 8 positions x 4096 vocab, and a
core only has 8 batches x 8 positions = 64 indices per window). Calls round-
robin over the 4 SWDGE queues; per-queue desc-gen serializes at ~1.8us fixed
+ 6ns/idx per call, so the gather phase is ~8 rounds x ~2.2us ~= 18us,
overlapped with the DVE slot folds. The point of this layout: NO collective.
The previous T-sharded version needed a [64,256] ReduceScatter whose
collectives-init barrier + ncfw wake + RS cost 95+us of a 115-137us exec
(the barrier absorbs cross-core NEFF launch skew, 33-81us measured); with no
collective in the NEFF every core runs independently and the whole kernel is
gather-bound.

Index layout (device gathers gq[P, k, :] = W1win[idx_i], i = idx-list pos):
  call for window w (queue w%4, slot k=w//4) gathers 64 rows to partitions
  P = i = j*8 + b_local (j = position-in-window, b_local = batch-in-core), so
  partition P always holds batch P%8 and (slot, queue, P//8) enumerate the 32
  windows x 8 positions. idx list position i lives at idx_tile[i%16, i//16]
  (16-partition wrap, replicated x8 for the 8 Q7 cores), 4 columns per window.
  Window-local row value = j*4096 + gate[8m + b_local, 8w + j]; the +j*4096
  rebase is done on device (ubias const + DVE add); the host only permutes/
  retypes gate_seq (value-independent layout marshaling).

Reduce: partitions [0:64] only (64..127 never written -> never read, no
memset): fold the 4 queue-tiles' 8 slots pairwise (3 DVE adds) to [64,8,256],
fold 8->4, then 4 accumulating mask matmuls (mask[p,b] = p%8==b) into a
f32 PSUM [8,256]. Tail MLP (relu + 2 matmul layers) runs per-core on its
[8,256] shard exactly as in the T-sharded version; host concatenates the
per-core [8,256] outputs.
"""

import sys

import numpy as np

if "/opt/trn_rl_repo" not in sys.path:
    sys.path.insert(0, "/opt/trn_rl_repo")

B = 64
T = 256
V = 4096
HID = 256
OUT = 256
NCORES = 8
BPC = B // NCORES          # batches per core = 8
WIN_POS = 8                # positions per gather window (int16 limit: 8*4096 = 32768 rows)
NWIN = T // WIN_POS        # 32 windows per core
WIN_ROWS = WIN_POS * V     # 32768
NIDX = BPC * WIN_POS       # 64 indices per window
NQ = 4                     # SWDGE queues
SLOTS = NWIN // NQ         # 8 windows per queue-tile

_CACHE = {}


def _host_consts():
    # idx list position i = (col%4)*16 + p%16; j = i//8 -> rebase j*4096 (int16)
    p = np.arange(128)[:, None]
    col = np.arange(NWIN * 4)[None, :]
    i = (col % 4) * 16 + (p % 16)
    ubias = np.broadcast_to((i // BPC) * V, (128, NWIN * 4)).astype(np.int16)
    # mask[p, b] = 1 if p % 8 == b   (partitions 0..63)
    import ml_dtypes
    mask = (np.arange(64)[:, None] % BPC == np.arange(BPC)[None, :]).astype(
        ml_dtypes.bfloat16
    )
    eye8 = np.eye(8, dtype=np.float32)
    eye8b = np.eye(8, dtype=ml_dtypes.bfloat16)
    return (
        np.ascontiguousarray(ubias),
        np.ascontiguousarray(mask),
        eye8,
        eye8b,
    )


def _build_nc():
    import concourse.bacc as bacc
    import concourse.mybir as mybir
    import concourse.tile as tile

    f32 = mybir.dt.float32
    bf16 = mybir.dt.bfloat16
    i16 = mybir.dt.int16
    Relu = mybir.ActivationFunctionType.Relu
    add = mybir.AluOpType.add

    ubias_np, mask_np, eye8_np, eye8b_np = _host_consts()

    nc = bacc.Bacc(
        "TRN2",
        target_bir_lowering=False,
        debug=False,
        num_devices=NCORES,
        num_swdge_queues=NQ,
    )

    gate_prep_d = nc.dram_tensor("gate_prep", [128, NWIN * 4], i16, kind="ExternalInput")
    w1_d = nc.dram_tensor("w1", [T * V, HID], bf16, kind="ExternalInput")
    w2_d = nc.dram_tensor("w2", [HID, HID], bf16, kind="ExternalInput")
    w3_d = nc.dram_tensor("w3", [HID, OUT], bf16, kind="ExternalInput")
    b1_d = nc.dram_tensor("b1t", [128, 2], f32, kind="ExternalInput")
    b2_d = nc.dram_tensor("b2", [1, HID], bf16, kind="ExternalInput")
    b3_d = nc.dram_tensor("b3", [1, OUT], bf16, kind="ExternalInput")
    out_d = nc.dram_tensor("out", [BPC, OUT], f32, kind="ExternalOutput")

    ubias_d = nc.inline_tensor(ubias_np, name="ubias_const")
    mask_d = nc.inline_tensor(mask_np, name="mask_const")
    eyeb_d = nc.inline_tensor(eye8b_np, name="eyeb_const")
    eye_d = nc.inline_tensor(eye8_np, name="eye_const")

    # Issue the mlp ucode library load before any Tile-scheduled work so the
    # ~10us Q7 library fetch overlaps the NEFF prologue instead of stalling
    # the first dma_gather until ~17us.
    from concourse import library_config

    nc.gpsimd.load_library(library_config.mlp)

    with tile.TileContext(nc) as tc:
        with (
            tc.tile_pool(name="const", bufs=1) as const,
            tc.tile_pool(name="gat", bufs=1) as gat,
            tc.tile_pool(name="work", bufs=2) as work,
            tc.tile_pool(name="psum", bufs=1, space="PSUM") as psum,
        ):
            # ---- critical path: indices ----
            gp = const.tile([128, NWIN * 4], i16, tag="gp")
            nc.sync.dma_start(gp[:], gate_prep_d[:])
            ub = const.tile([128, NWIN * 4], i16, tag="ub")
            nc.sync.dma_start(ub[:], ubias_d[:])
            idx = const.tile([128, NWIN * 4], i16, tag="idx")
            nc.vector.tensor_tensor(idx[:], gp[:], ub[:], add)

            # ---- gathers (SWDGE custom ucode, round-robin over 4 queues) ----
            g_tiles = []
            for q in range(NQ):
                g = gat.tile([128, SLOTS, HID], bf16, tag=f"g{q}")
                g_tiles.append(g)
            for w in range(NWIN):
                q, k = w % NQ, w // NQ
                nc.gpsimd.dma_gather(
                    g_tiles[q][:, k : k + 1, :],
                    w1_d[w * WIN_ROWS : (w + 1) * WIN_ROWS, :],
                    idx[:, w * 4 : (w + 1) * 4],
                    NIDX,
                    NIDX,
                    HID,
                    queue_num=q,
                )

            # ---- constants / weights preload (no deps; fills DMA idle time) ----
            mask_sb = const.tile([64, BPC], bf16, tag="mask")
            nc.scalar.dma_start(mask_sb[:], mask_d[:])
            eyeb_sb = const.tile([8, 8], bf16, tag="eyeb")
            nc.scalar.dma_start(eyeb_sb[:], eyeb_d[:])
            eye_sb = const.tile([8, 8], f32, tag="eye")
            nc.scalar.dma_start(eye_sb[:], eye_d[:])
            w2_sb = const.tile([128, 2, HID], bf16, tag="w2")
            nc.scalar.dma_start(w2_sb[:], w2_d[:, :].rearrange("(k p) n -> p k n", p=128))
            w3_sb = const.tile([128, 2, OUT], bf16, tag="w3")
            nc.scalar.dma_start(w3_sb[:], w3_d[:, :].rearrange("(k p) n -> p k n", p=128))
            b1_sb = const.tile([128, 2], f32, tag="b1")
            nc.scalar.dma_start(b1_sb[:], b1_d[:])
            b2_sb = const.tile([1, HID], bf16, tag="b2")
            nc.scalar.dma_start(b2_sb[:], b2_d[:])
            b3_sb = const.tile([1, OUT], bf16, tag="b3")
            nc.scalar.dma_start(b3_sb[:], b3_d[:])
            ones8 = const.tile([1, BPC], bf16, tag="ones8")
            nc.vector.memset(ones8[:], 1.0)

            # ---- slot folds (DVE, partitions 0..63 only) + mask matmuls ----
            u1 = work.tile([64, SLOTS, HID], bf16, tag="u1")
            nc.vector.tensor_add(u1[:], g_tiles[0][0:64, :, :], g_tiles[1][0:64, :, :])
            u2 = work.tile([64, SLOTS, HID], bf16, tag="u2")
            nc.vector.tensor_add(u2[:], g_tiles[2][0:64, :, :], g_tiles[3][0:64, :, :])
            u3 = work.tile([64, SLOTS, HID], bf16, tag="u3")
            nc.vector.tensor_add(u3[:], u1[:], u2[:])
            u4 = work.tile([64, SLOTS // 2, HID], bf16, tag="u4")
            nc.vector.tensor_add(u4[:], u3[:, 0 : SLOTS // 2, :], u3[:, SLOTS // 2 :, :])

            psum_part = psum.tile([BPC, HID], f32, tag="part")
            for k in range(SLOTS // 2):
                nc.tensor.matmul(
                    psum_part[:],
                    mask_sb[:],
                    u4[:, k, :],
                    start=(k == 0),
                    stop=(k == SLOTS // 2 - 1),
                )
            h_sb = work.tile([BPC, HID], f32, tag="h")
            nc.vector.tensor_copy(h_sb[:], psum_part[:])

            # ---- tail MLP on [8, 256] shard ----
            # hT = transpose(h) (PE), relu(hT + b1T) per 128-chunk
            hTr = []
            for m in range(2):
                p_hT = psum.tile([128, BPC], f32, tag=f"p_hT{m}")
                nc.tensor.transpose(p_hT[:], h_sb[:, m * 128 : (m + 1) * 128], eye_sb[:])
                t = work.tile([128, BPC], bf16, tag=f"hTr{m}")
                nc.scalar.activation(t[:], p_hT[:], Relu, bias=b1_sb[:, m : m + 1])
                hTr.append(t)

            # h2 = relu(hTr.T @ W2 + b2)   -> [8, 256]
            p_h2 = psum.tile([BPC, HID], f32, tag="p_h2")
            nc.tensor.matmul(p_h2[:], hTr[0][:], w2_sb[:, 0, :], start=True, stop=False)
            nc.tensor.matmul(p_h2[:], hTr[1][:], w2_sb[:, 1, :], start=False, stop=False)
            nc.tensor.matmul(p_h2[:], ones8[:], b2_sb[:], start=False, stop=True)
            h2_sb = work.tile([BPC, HID], bf16, tag="h2")
            nc.scalar.activation(h2_sb[:], p_h2[:], Relu)

            # out = h2 @ W3 + b3          -> [8, 256]
            h2T = []
            for m in range(2):
                p_h2T = psum.tile([128, BPC], bf16, tag=f"p_h2T{m}")
                nc.tensor.transpose(p_h2T[:], h2_sb[:, m * 128 : (m + 1) * 128], eyeb_sb[:])
                t = work.tile([128, BPC], bf16, tag=f"h2T{m}")
                nc.vector.tensor_copy(t[:], p_h2T[:])
                h2T.append(t)
            p_o = psum.tile([BPC, OUT], f32, tag="p_o")
            nc.tensor.matmul(p_o[:], h2T[0][:], w3_sb[:, 0, :], start=True, stop=False)
            nc.tensor.matmul(p_o[:], h2T[1][:], w3_sb[:, 1, :], start=False, stop=False)
            nc.tensor.matmul(p_o[:], ones8[:], b3_sb[:], start=False, stop=True)
            out_sb = work.tile([BPC, OUT], f32, tag="out_sb")
            nc.vector.tensor_copy(out_sb[:], p_o[:])
            nc.sync.dma_start(out_d[:], out_sb[:])

    nc.compile()
    return nc


def get_nc():
    if "nc" not in _CACHE:
        _CACHE["nc"] = _build_nc()
    return _CACHE["nc"]


def make_in_maps(gate_seq, W1, b1, W2, b2, W3, b3):
    """Shard/marshal the full inputs into per-core input maps (values untouched:
    pure slicing, transposition, retyping and tiling)."""
    gate_seq = np.asarray(gate_seq)
    import ml_dtypes

    W1 = np.ascontiguousarray(np.asarray(W1).astype(ml_dtypes.bfloat16))
    W2 = np.ascontiguousarray(np.asarray(W2).astype(ml_dtypes.bfloat16))
    W3 = np.ascontiguousarray(np.asarray(W3).astype(ml_dtypes.bfloat16))
    b1 = np.asarray(b1, dtype=np.float32)
    b2 = np.asarray(b2, dtype=np.float32)
    b3 = np.asarray(b3, dtype=np.float32)

    b1t = np.ascontiguousarray(b1.reshape(2, 128).T)  # b1t[p, m] = b1[m*128 + p]
    b2r = np.ascontiguousarray(b2[None, :].astype(ml_dtypes.bfloat16))
    b3r = np.ascontiguousarray(b3[None, :].astype(ml_dtypes.bfloat16))

    # index-layout permutation (see module docstring):
    # gate_prep[p16, col] = gate_seq[8m + i%8, 8*(col//4) + i//8], i = (col%4)*16 + p16
    p16 = np.arange(16)[:, None]                     # [16, 1]
    col = np.arange(NWIN * 4)[None, :]               # [1, 128]
    i = (col % 4) * 16 + p16                         # [16, 128]
    b_idx = i % BPC
    t_idx = np.broadcast_to((col // 4) * WIN_POS + i // BPC, b_idx.shape)

    in_maps = []
    for m in range(NCORES):
        gs = gate_seq[m * BPC : (m + 1) * BPC, :]    # [8, 256]
        A = gs[b_idx, t_idx].astype(np.int16)        # [16, 128]
        gate_prep = np.ascontiguousarray(np.tile(A, (8, 1)))  # [128, 128]
        in_maps.append(
            {
                "gate_prep": gate_prep,
                "w1": W1,
                "w2": W2,
                "w3": W3,
                "b1t": b1t,
                "b2": b2r,
                "b3": b3r,
            }
        )
    return in_maps


def run(inputs, trace=False, **spmd_kwargs):
    from concourse.bass_utils import run_bass_kernel_spmd

    nc = get_nc()
    in_maps = make_in_maps(**inputs)
    res = run_bass_kernel_spmd(
        nc, in_maps, core_ids=list(range(NCORES)), trace=trace, **spmd_kwargs
    )
    out = np.concatenate([r["out"] for r in res.results], axis=0)
    return out, res


def kernel(**inputs) -> np.ndarray:
    out, _ = run(inputs, trace=False)
    return out
